# revision 3
# baseline (speedup 1.0000x reference)
"""Distributed causal attention + RoPE for trn2 (8 NeuronCores).

Sharding: batch (2) x head-groups (4 heads/core). Core c: batch c//4,
heads 4*(c%4)..4*(c%4)+3. Attention computed in S^T layout
([k_part, q_free]) so no on-device transposes are needed; softmax sums
come from a ones-vector matmul over partitions. Out-projection is
column-parallel after an intra-group AllGather of the per-core
attention outputs.
"""
import sys
for _p in ('/opt/trn_rl_repo',):
    if _p not in sys.path:
        sys.path.insert(0, _p)

from contextlib import ExitStack

import numpy as np
import ml_dtypes

B, S, H, NH, HD = 2, 2048, 2048, 16, 128
HPC = 4            # heads per core
DH = HPC * HD      # 512 local dims
QC = 512           # q-chunk width (attention + AG round)
SCALE = HD ** -0.5

_cached = {}


def _build(reps=1):
    import concourse.bacc as bacc
    import concourse.mybir as mybir
    import concourse.tile as tile

    F32 = mybir.dt.float32
    BF = mybir.dt.bfloat16
    AF = mybir.ActivationFunctionType
    ALU = mybir.AluOpType

    nc = bacc.Bacc("TRN2", target_bir_lowering=False, debug=False, num_devices=8)
    xT_d = nc.dram_tensor("xT", [H, S], BF, kind="ExternalInput").ap()
    wqT_d = nc.dram_tensor("wqT", [H, DH], BF, kind="ExternalInput").ap()
    wkT_d = nc.dram_tensor("wkT", [H, DH], BF, kind="ExternalInput").ap()
    wvT_d = nc.dram_tensor("wvT", [H, DH], BF, kind="ExternalInput").ap()
    woT_d = nc.dram_tensor("woT", [H, DH], BF, kind="ExternalInput").ap()
    cosT_d = nc.dram_tensor("cosT", [HD, S], F32, kind="ExternalInput").ap()
    sinTs_d = nc.dram_tensor("sinTs", [HD, S], F32, kind="ExternalInput").ap()
    mask_d = nc.dram_tensor("mask01", [128, 128], BF, kind="ExternalInput").ap()
    out_d = nc.dram_tensor("out", [S, DH], F32, kind="ExternalOutput").ap()

    EB = H // 128     # 16 contraction blocks
    n_sc = S // QC    # 4 s-chunks

    with ExitStack() as ctx:
        tc = ctx.enter_context(tile.TileContext(nc))
        wpool = ctx.enter_context(tc.tile_pool(name="wpool", bufs=3))
        wop = ctx.enter_context(tc.tile_pool(name="wo", bufs=1))
        xp = ctx.enter_context(tc.tile_pool(name="xp", bufs=2))
        cp = ctx.enter_context(tc.tile_pool(name="consts", bufs=1))
        qkp = ctx.enter_context(tc.tile_pool(name="qk", bufs=1))
        vp = ctx.enter_context(tc.tile_pool(name="vp", bufs=1))
        rp = ctx.enter_context(tc.tile_pool(name="rope", bufs=2))
        atp = ctx.enter_context(tc.tile_pool(name="at", bufs=4))
        otp = ctx.enter_context(tc.tile_pool(name="ot", bufs=4))
        rnp = ctx.enter_context(tc.tile_pool(name="rn", bufs=2))
        ocp = ctx.enter_context(tc.tile_pool(name="oc", bufs=2))
        ppA = ctx.enter_context(tc.tile_pool(name="ppA", bufs=2, space="PSUM"))
        ppS = ctx.enter_context(tc.tile_pool(name="ppS", bufs=2, space="PSUM"))
        ppO = ctx.enter_context(tc.tile_pool(name="ppO", bufs=2, space="PSUM"))
        ppR = ctx.enter_context(tc.tile_pool(name="ppR", bufs=2, space="PSUM"))
        dramp = ctx.enter_context(tc.tile_pool(name="dramp", bufs=2, space="DRAM"))

        # ---- constants / weights ----
        wq_sb = wpool.tile([128, EB, DH], BF, tag="w", name="wq_sb")
        wk_sb = wpool.tile([128, EB, DH], BF, tag="w", name="wk_sb")
        wv_sb = wpool.tile([128, EB, DH], BF, tag="w", name="wv_sb")
        nc.sync.dma_start(out=wq_sb[:], in_=wqT_d.rearrange("(e p) d -> p e d", p=128))
        nc.sync.dma_start(out=wk_sb[:], in_=wkT_d.rearrange("(e p) d -> p e d", p=128))
        nc.sync.dma_start(out=wv_sb[:], in_=wvT_d.rearrange("(e p) d -> p e d", p=128))
        wo_sb = wop.tile([128, EB, DH], BF, tag="wo", name="wo_sb")
        nc.sync.dma_start(out=wo_sb[:], in_=woT_d.rearrange("(e p) d -> p e d", p=128))
        cos_sb = cp.tile([HD, S], F32, tag="cos", name="cos_sb")
        sin_sb = cp.tile([HD, S], F32, tag="sin", name="sin_sb")
        nc.sync.dma_start(out=cos_sb[:], in_=cosT_d[:])
        nc.sync.dma_start(out=sin_sb[:], in_=sinTs_d[:])
        mask_sb = cp.tile([128, 128], BF, tag="mask", name="mask_sb")
        nc.sync.dma_start(out=mask_sb[:], in_=mask_d[:])
        ones_sb = cp.tile([128, 1], BF, tag="ones", name="ones_sb")
        nc.vector.memset(ones_sb[:], 1.0)

        qT = [qkp.tile([HD, S], BF, tag=f"qT{h}", name=f"qT{h}") for h in range(HPC)]
        kT = [qkp.tile([HD, S], BF, tag=f"kT{h}", name=f"kT{h}") for h in range(HPC)]
        v_sb = vp.tile([128, S // 128, DH], BF, tag="v", name="v_sb")

        def proj_chunk(sc):
            s0 = sc * QC
            xt = xp.tile([128, EB, QC], BF, tag="xt", name="xt")
            nc.sync.dma_start(
                out=xt[:],
                in_=xT_d.rearrange("(e p) s -> p e s", p=128)[:, :, s0:s0 + QC])
            for h in range(HPC):
                d0 = h * HD
                for (wsb, dstT) in ((wq_sb, qT[h]), (wk_sb, kT[h])):
                    ps = ppA.tile([128, QC], F32, tag="pA", name="ps")
                    for e in range(EB):
                        nc.tensor.matmul(ps[:], wsb[:, e, d0:d0 + HD], xt[:, e, :],
                                         start=(e == 0), stop=(e == EB - 1))
                    m1 = rp.tile([128, QC], F32, tag="m1", name="m1")
                    m2 = rp.tile([128, QC], F32, tag="m2", name="m2")
                    nc.vector.tensor_tensor(m2[0:64, :], ps[64:128, :], sin_sb[0:64, s0:s0 + QC], op=ALU.mult)
                    nc.vector.tensor_tensor(m2[64:128, :], ps[0:64, :], sin_sb[64:128, s0:s0 + QC], op=ALU.mult)
                    nc.vector.tensor_tensor(m1[:], ps[:], cos_sb[:, s0:s0 + QC], op=ALU.mult)
                    nc.vector.tensor_tensor(dstT[:, s0:s0 + QC], m1[:], m2[:], op=ALU.add)
            # V for this s-chunk: natural [s, d] layout
            for ss in range(QC // 128):
                sb = (s0 // 128) + ss
                ps = ppA.tile([128, DH], F32, tag="pA", name="psv")
                for e in range(EB):
                    nc.tensor.matmul(ps[:], xt[:, e, ss * 128:(ss + 1) * 128], wv_sb[:, e, :],
                                     start=(e == 0), stop=(e == EB - 1))
                nc.scalar.copy(v_sb[:, sb, :], ps[:])

        def attn_chunk(qc, h, agin):
            q0 = qc * QC
            nkb = (q0 + QC) // 128  # causal: k blocks up to chunk end
            ot_ps = ppO.tile([128, QC], F32, tag="pO", name="ot_ps")
            sums = ppR.tile([1, QC], F32, tag="pR", name="sums")
            for kb in range(nkb):
                dj = kb - q0 // 128   # >=0 on diagonal chunk
                o = dj * 128 if dj >= 0 else 0
                sps = ppS.tile([128, QC], F32, tag="pS", name="sps")
                nc.tensor.matmul(sps[:, o:QC], kT[h][:, kb * 128:(kb + 1) * 128],
                                 qT[h][:, q0 + o:q0 + QC],
                                 start=True, stop=True)
                at = atp.tile([128, QC], BF, tag="at", name="at")
                nc.scalar.activation(at[:, o:QC], sps[:, o:QC], AF.Exp, scale=SCALE)
                if dj >= 0:
                    nc.vector.tensor_tensor(at[:, o:o + 128], at[:, o:o + 128], mask_sb[:], op=ALU.mult)
                nc.tensor.matmul(ot_ps[:, o:QC], v_sb[:, kb, h * HD:(h + 1) * HD], at[:, o:QC],
                                 start=(kb == 0), stop=(kb == nkb - 1), skip_group_check=True)
                nc.tensor.matmul(sums[:, o:QC], ones_sb[:], at[:, o:QC],
                                 start=(kb == 0), stop=(kb == nkb - 1), skip_group_check=True)
            recip = rnp.tile([1, QC], F32, tag="recip", name="recip")
            nc.vector.reciprocal(recip[:], sums[:])
            rbc = rnp.tile([128, QC], F32, tag="rbc", name="rbc")
            nc.gpsimd.partition_broadcast(rbc[:], recip[:])
            ot = otp.tile([128, QC], BF, tag="ot", name="ot")
            nc.vector.tensor_tensor(ot[:], ot_ps[:], rbc[:], op=ALU.mult)
            nc.sync.dma_start(out=agin[h * 128:(h + 1) * 128, :], in_=ot[:])

        def out_proj(ago, qc):
            ag_sb = wpool.tile([128, EB, QC], BF, tag="w", name="ag_sb")
            nc.sync.dma_start(out=ag_sb[:], in_=ago.rearrange("(e p) q -> p e q", p=128))
            for qs in range(QC // 128):
                ps = ppA.tile([128, DH], F32, tag="pA", name="pso")
                for e in range(EB):
                    nc.tensor.matmul(ps[:], ag_sb[:, e, qs * 128:(qs + 1) * 128], wo_sb[:, e, :],
                                     start=(e == 0), stop=(e == EB - 1))
                oc = ocp.tile([128, DH], F32, tag="oc", name="oc")
                nc.scalar.copy(oc[:], ps[:])
                nc.sync.dma_start(out=out_d[qc * QC + qs * 128: qc * QC + (qs + 1) * 128, :], in_=oc[:])

        for _rep in range(reps):
            for sc in range(n_sc):
                proj_chunk(sc)
            agout = {}
            for qc in range(n_sc):
                agin = dramp.tile([DH, QC], BF, tag="agin", name="agin")
                for h in range(HPC):
                    attn_chunk(qc, h, agin)
                ago = dramp.tile([H, QC], BF, tag="agout", name="ago")
                agout[qc] = ago
                nc.gpsimd.collective_compute(
                    "AllGather", mybir.AluOpType.bypass,
                    ins=[agin[:]], outs=[ago[:]],
                    replica_groups=[[0, 1, 2, 3], [4, 5, 6, 7]],
                )
                if qc >= 1:
                    out_proj(agout[qc - 1], qc - 1)
            out_proj(agout[n_sc - 1], n_sc - 1)

    nc.compile()
    return nc


def _prep_in_maps(hidden_states, cos, sin, Wq, Wk, Wv, Wo):
    bf = ml_dtypes.bfloat16
    cosT = np.ascontiguousarray(cos[0, 0].T).astype(np.float32)
    sinTs = np.ascontiguousarray(sin[0, 0].T).astype(np.float32).copy()
    sinTs[0:64] *= -1.0
    mask01 = np.triu(np.ones((128, 128), np.float32)).astype(bf)
    in_maps = []
    for c in range(8):
        b, t = c // 4, c % 4
        rows = slice(DH * t, DH * (t + 1))
        in_maps.append({
            "xT": np.ascontiguousarray(hidden_states[b].T).astype(bf),
            "wqT": np.ascontiguousarray(Wq[rows, :].T).astype(bf),
            "wkT": np.ascontiguousarray(Wk[rows, :].T).astype(bf),
            "wvT": np.ascontiguousarray(Wv[rows, :].T).astype(bf),
            "woT": np.ascontiguousarray(Wo[rows, :].T).astype(bf),
            "cosT": cosT,
            "sinTs": sinTs,
            "mask01": mask01,
        })
    return in_maps


def kernel(hidden_states, cos, sin, Wq, Wk, Wv, Wo):
    from concourse.bass_utils import run_bass_kernel_spmd
    if "nc" not in _cached:
        _cached["nc"] = _build()
    nc = _cached["nc"]
    in_maps = _prep_in_maps(hidden_states, cos, sin, Wq, Wk, Wv, Wo)
    res = run_bass_kernel_spmd(nc, in_maps, core_ids=list(range(8)))
    out = np.empty((B, S, H), np.float32)
    for c in range(8):
        b, t = c // 4, c % 4
        out[b, :, DH * t:DH * (t + 1)] = res.results[c]["out"]
    return out


# revision 12
# speedup vs baseline: 445.0389x; 445.0389x over previous
"""Distributed causal attention + RoPE for trn2 (8 NeuronCores).

Sharding: batch (2) x head-groups (4 heads/core). Core c: batch c//4,
heads 4*(c%4)..4*(c%4)+3. Attention computed in S^T layout
([k_part, q_free]) so no on-device transposes are needed; softmax sums
come from a ones-vector matmul over partitions. Out-projection is
column-parallel after an intra-group AllGather of the per-core
attention outputs.
"""
import sys
for _p in ('/opt/trn_rl_repo',):
    if _p not in sys.path:
        sys.path.insert(0, _p)

from contextlib import ExitStack

import numpy as np
import ml_dtypes

B, S, H, NH, HD = 2, 2048, 2048, 16, 128
HPC = 4            # heads per core
DH = HPC * HD      # 512 local dims
QC = 512           # q-chunk width (attention + AG round)
SCALE = HD ** -0.5

_cached = {}


def _build(reps=1, feats=frozenset({'attn','norm','cc','outproj'}), hw_loop=0):
    import concourse.bacc as bacc
    import concourse.mybir as mybir
    import concourse.tile as tile

    F32 = mybir.dt.float32
    BF = mybir.dt.bfloat16
    AF = mybir.ActivationFunctionType
    ALU = mybir.AluOpType

    nc = bacc.Bacc("TRN2", target_bir_lowering=False, debug=False, num_devices=8)
    xT_d = nc.dram_tensor("xT", [H, S], BF, kind="ExternalInput").ap()
    wqT_d = nc.dram_tensor("wqT", [H, DH], BF, kind="ExternalInput").ap()
    wkT_d = nc.dram_tensor("wkT", [H, DH], BF, kind="ExternalInput").ap()
    wvT_d = nc.dram_tensor("wvT", [H, DH], BF, kind="ExternalInput").ap()
    woT_d = nc.dram_tensor("woT", [H, DH], BF, kind="ExternalInput").ap()
    cosT_d = nc.dram_tensor("cosT", [HD, S], BF, kind="ExternalInput").ap()
    sinTs_d = nc.dram_tensor("sinTs", [HD, S], BF, kind="ExternalInput").ap()
    mask_d = nc.dram_tensor("mask01", [128, 128], BF, kind="ExternalInput").ap()
    out_d = nc.dram_tensor("out", [S, DH], F32, kind="ExternalOutput").ap()

    EB = H // 128     # 16 contraction blocks
    n_sc = S // QC    # 4 s-chunks

    with ExitStack() as ctx:
        tc = ctx.enter_context(tile.TileContext(nc))
        wpool = ctx.enter_context(tc.tile_pool(name="wpool", bufs=3))
        agp = ctx.enter_context(tc.tile_pool(name="agp", bufs=2))
        wop = ctx.enter_context(tc.tile_pool(name="wo", bufs=1))
        xp = ctx.enter_context(tc.tile_pool(name="xp", bufs=2))
        cp = ctx.enter_context(tc.tile_pool(name="consts", bufs=1))
        qkp = ctx.enter_context(tc.tile_pool(name="qk", bufs=1))
        vp = ctx.enter_context(tc.tile_pool(name="vp", bufs=1))
        rp = ctx.enter_context(tc.tile_pool(name="rope", bufs=2))
        atp = ctx.enter_context(tc.tile_pool(name="at", bufs=3))
        otp = ctx.enter_context(tc.tile_pool(name="ot", bufs=2))
        rnp = ctx.enter_context(tc.tile_pool(name="rn", bufs=2))
        ocp = ctx.enter_context(tc.tile_pool(name="oc", bufs=1))
        ppA = ctx.enter_context(tc.tile_pool(name="ppA", bufs=2, space="PSUM"))
        ppS = ctx.enter_context(tc.tile_pool(name="ppS", bufs=2, space="PSUM"))
        ppO = ctx.enter_context(tc.tile_pool(name="ppO", bufs=2, space="PSUM"))
        ppR = ctx.enter_context(tc.tile_pool(name="ppR", bufs=2, space="PSUM"))
        dramp = ctx.enter_context(tc.tile_pool(name="dramp", bufs=2, space="DRAM"))

        # ---- constants / weights ----
        wvars = {}

        def load_qkv_weights():
            wq_sb = wpool.tile([128, EB, DH], BF, tag="w", name="wq_sb")
            wk_sb = wpool.tile([128, EB, DH], BF, tag="w", name="wk_sb")
            wv_sb = wpool.tile([128, EB, DH], BF, tag="w", name="wv_sb")
            nc.sync.dma_start(out=wq_sb[:], in_=wqT_d.rearrange("(e p) d -> p e d", p=128))
            nc.sync.dma_start(out=wk_sb[:], in_=wkT_d.rearrange("(e p) d -> p e d", p=128))
            nc.sync.dma_start(out=wv_sb[:], in_=wvT_d.rearrange("(e p) d -> p e d", p=128))
            wvars["wq"], wvars["wk"], wvars["wv"] = wq_sb, wk_sb, wv_sb

        wo_sb = wop.tile([128, EB, DH], BF, tag="wo", name="wo_sb")
        nc.sync.dma_start(out=wo_sb[:], in_=woT_d.rearrange("(e p) d -> p e d", p=128))
        cos_sb = cp.tile([HD, S], BF, tag="cos", name="cos_sb")
        sin_sb = cp.tile([HD, S], BF, tag="sin", name="sin_sb")
        nc.sync.dma_start(out=cos_sb[:], in_=cosT_d[:])
        nc.sync.dma_start(out=sin_sb[:], in_=sinTs_d[:])
        mask_sb = cp.tile([128, 128], BF, tag="mask", name="mask_sb")
        nc.sync.dma_start(out=mask_sb[:], in_=mask_d[:])
        ones_sb = cp.tile([128, 1], BF, tag="ones", name="ones_sb")
        nc.vector.memset(ones_sb[:], 1.0)

        qT = [qkp.tile([HD, S], BF, tag=f"qT{h}", name=f"qT{h}") for h in range(HPC)]
        kT = [qkp.tile([HD, S], BF, tag=f"kT{h}", name=f"kT{h}") for h in range(HPC)]
        v_sb = vp.tile([128, S // 128, DH], BF, tag="v", name="v_sb")

        def proj_chunk(sc):
            s0 = sc * QC
            xt = xp.tile([128, EB, QC], BF, tag="xt", name="xt")
            nc.sync.dma_start(
                out=xt[:],
                in_=xT_d.rearrange("(e p) s -> p e s", p=128)[:, :, s0:s0 + QC])
            for h in range(HPC):
                d0 = h * HD
                for (wsb, dstT) in ((wvars["wq"], qT[h]), (wvars["wk"], kT[h])):
                    ps = ppA.tile([128, QC], F32, tag="pA", name="ps")
                    for e in range(EB):
                        nc.tensor.matmul(ps[:], wsb[:, e, d0:d0 + HD], xt[:, e, :],
                                         start=(e == 0), stop=(e == EB - 1))
                    m1 = rp.tile([128, QC], F32, tag="m1", name="m1")
                    m2 = rp.tile([128, QC], F32, tag="m2", name="m2")
                    nc.vector.tensor_tensor(m2[0:64, :], ps[64:128, :], sin_sb[0:64, s0:s0 + QC], op=ALU.mult)
                    nc.vector.tensor_tensor(m2[64:128, :], ps[0:64, :], sin_sb[64:128, s0:s0 + QC], op=ALU.mult)
                    nc.vector.tensor_tensor(m1[:], ps[:], cos_sb[:, s0:s0 + QC], op=ALU.mult)
                    nc.vector.tensor_tensor(dstT[:, s0:s0 + QC], m1[:], m2[:], op=ALU.add)
            # V for this s-chunk: natural [s, d] layout
            for ss in range(QC // 128):
                sb = (s0 // 128) + ss
                ps = ppA.tile([128, DH], F32, tag="pA", name="psv")
                for e in range(EB):
                    nc.tensor.matmul(ps[:], xt[:, e, ss * 128:(ss + 1) * 128], wvars["wv"][:, e, :],
                                     start=(e == 0), stop=(e == EB - 1))
                nc.scalar.copy(v_sb[:, sb, :], ps[:])

        def attn_chunk(qc, h, agin):
            q0 = qc * QC
            if 'attn' not in feats:
                ot = otp.tile([128, QC], BF, tag="ot", name="ot")
                nc.scalar.copy(ot[:], v_sb[:, qc, :])
                nc.sync.dma_start(out=agin[h * 128:(h + 1) * 128, :], in_=ot[:])
                return
            nkb = (q0 + QC) // 128  # causal: k blocks up to chunk end
            ot_ps = ppO.tile([128, QC], F32, tag="pO", name="ot_ps")
            sums = ppR.tile([1, QC], F32, tag="pR", name="sums")
            for kb in range(nkb):
                dj = kb - q0 // 128   # >=0 on diagonal chunk
                o = dj * 128 if dj >= 0 else 0
                sps = ppS.tile([128, QC], F32, tag="pS", name="sps")
                nc.tensor.matmul(sps[:, o:QC], kT[h][:, kb * 128:(kb + 1) * 128],
                                 qT[h][:, q0 + o:q0 + QC],
                                 start=True, stop=True)
                at = atp.tile([128, QC], BF, tag="at", name="at")
                nc.scalar.activation(at[:, o:QC], sps[:, o:QC], AF.Exp, scale=SCALE)
                if dj >= 0:
                    nc.vector.tensor_tensor(at[:, o:o + 128], at[:, o:o + 128], mask_sb[:], op=ALU.mult)
                nc.tensor.matmul(ot_ps[:, o:QC], v_sb[:, kb, h * HD:(h + 1) * HD], at[:, o:QC],
                                 start=(kb == 0), stop=(kb == nkb - 1), skip_group_check=True)
                nc.tensor.matmul(sums[:, o:QC], ones_sb[:], at[:, o:QC],
                                 start=(kb == 0), stop=(kb == nkb - 1), skip_group_check=True)
            ot = otp.tile([128, QC], BF, tag="ot", name="ot")
            if 'norm' in feats:
                recip = rnp.tile([1, QC], F32, tag="recip", name="recip")
                nc.vector.reciprocal(recip[:], sums[:])
                rbc = rnp.tile([128, QC], F32, tag="rbc", name="rbc")
                nc.gpsimd.partition_broadcast(rbc[:], recip[:])
                nc.vector.tensor_tensor(ot[:], ot_ps[:], rbc[:], op=ALU.mult)
            else:
                nc.scalar.copy(ot[:], ot_ps[:])
            nc.sync.dma_start(out=agin[h * 128:(h + 1) * 128, :], in_=ot[:])

        def out_proj(ago, qc):
            ag_sb = agp.tile([128, EB, QC], BF, tag="ag", name="ag_sb")
            nc.sync.dma_start(out=ag_sb[:], in_=ago.rearrange("(e p) q -> p e q", p=128))
            for qs in range(QC // 128):
                ps = ppA.tile([128, DH], F32, tag="pA", name="pso")
                for e in range(EB):
                    nc.tensor.matmul(ps[:], ag_sb[:, e, qs * 128:(qs + 1) * 128], wo_sb[:, e, :],
                                     start=(e == 0), stop=(e == EB - 1))
                oc = ocp.tile([128, DH], F32, tag="oc", name="oc")
                nc.scalar.copy(oc[:], ps[:])
                nc.sync.dma_start(out=out_d[qc * QC + qs * 128: qc * QC + (qs + 1) * 128, :], in_=oc[:])

        def body():
            # Interleaved: per s-chunk, project then immediately run attention
            # for that q-chunk (its K/V prefix is complete), then kick its
            # AllGather so it overlaps the next chunk's projection+attention.
            load_qkv_weights()
            agout = {}
            for qc in range(n_sc):
                proj_chunk(qc)
                agin = dramp.tile([DH, QC], BF, tag="agin", name="agin")
                for h in range(HPC):
                    attn_chunk(qc, h, agin)
                ago = dramp.tile([H, QC], BF, tag="agout", name="ago")
                agout[qc] = ago
                if 'cc' in feats:
                    nc.gpsimd.collective_compute(
                        "AllGather", mybir.AluOpType.bypass,
                        ins=[agin[:]], outs=[ago[:]],
                        replica_groups=[[0, 1, 2, 3], [4, 5, 6, 7]],
                    )
                else:
                    nc.sync.dma_start(out=ago[0:DH, :], in_=agin[:])
                if qc >= 1 and 'outproj' in feats:
                    out_proj(agout[qc - 1], qc - 1)
            if 'outproj' in feats:
                out_proj(agout[n_sc - 1], n_sc - 1)
            else:
                oc = ocp.tile([128, DH], F32, tag="oc", name="oc")
                nc.sync.dma_start(out=oc[:, 0:QC // 2], in_=agout[n_sc - 1][0:128, :].bitcast(F32))
                nc.sync.dma_start(out=out_d[0:128, :], in_=oc[:])

        if hw_loop:
            assert 'cc' not in feats, "collectives cannot sit inside a hw loop"
            with tc.For_i(0, hw_loop, 1):
                body()
        else:
            for _rep in range(reps):
                body()

    nc.compile()
    return nc


def _prep_in_maps(hidden_states, cos, sin, Wq, Wk, Wv, Wo):
    bf = ml_dtypes.bfloat16
    cosT = np.ascontiguousarray(cos[0, 0].T).astype(bf)
    sinTs = np.ascontiguousarray(sin[0, 0].T).astype(np.float32).copy()
    sinTs[0:64] *= -1.0
    sinTs = sinTs.astype(bf)
    mask01 = np.triu(np.ones((128, 128), np.float32)).astype(bf)
    in_maps = []
    for c in range(8):
        b, t = c // 4, c % 4
        rows = slice(DH * t, DH * (t + 1))
        in_maps.append({
            "xT": np.ascontiguousarray(hidden_states[b].T).astype(bf),
            "wqT": np.ascontiguousarray(Wq[rows, :].T).astype(bf),
            "wkT": np.ascontiguousarray(Wk[rows, :].T).astype(bf),
            "wvT": np.ascontiguousarray(Wv[rows, :].T).astype(bf),
            "woT": np.ascontiguousarray(Wo[rows, :].T).astype(bf),
            "cosT": cosT,
            "sinTs": sinTs,
            "mask01": mask01,
        })
    return in_maps


def kernel(hidden_states, cos, sin, Wq, Wk, Wv, Wo):
    from concourse.bass_utils import run_bass_kernel_spmd
    if "nc" not in _cached:
        _cached["nc"] = _build()
    nc = _cached["nc"]
    in_maps = _prep_in_maps(hidden_states, cos, sin, Wq, Wk, Wv, Wo)
    res = run_bass_kernel_spmd(nc, in_maps, core_ids=list(range(8)))
    out = np.empty((B, S, H), np.float32)
    for c in range(8):
        b, t = c // 4, c % 4
        out[b, :, DH * t:DH * (t + 1)] = res.results[c]["out"]
    return out
